# revision 1
# baseline (speedup 1.0000x reference)
"""Trainium2 Bass kernel for nn_ClassConfusionLoss.

Self-contained: takes FULL inputs pred (64,64,128,128) f32, gt (64,64,128,128) i32,
shards the spatial W axis across 8 NeuronCores, computes per-core partial weighted
covariance M (64x64, as a 128x128 PSUM block pair), reduces on host and applies the
final row-normalization + trace (O(C^2), negligible).

Math: the reference's global scalars num_pos and S = sum(n*w_raw) scale cov by
alpha = num_pos/S, which cancels in cov / cov.sum(axis=1). So only
M[c,k] = sum_p n_p*w_raw_p*x_pc*x_pk is needed, where x[b,c,w,h] =
pred[b,c,w,h]/(sum_c' pred[c,c',w,h] + eps)  (batch index c — valid since B == C),
n = sum_c(gt==1), w_raw = 1 + exp(-ent), ent = -sum_c x*log(x+eps).

Per core (w-slab of 16, processed as 8 adjacent-w pairs):
  pred_nat/gt_nat [(j*64+c)=128p, b=64, h=128] bf16   <- SWDGE cast DMA
  D/N/E[h, j, b] via lhsT-trick matmuls (chunk lhsT = nat[:, b, :], rhs = half-indicator)
  Rt[(j*64+b), h] = PE-transpose of 1/D ; x_nat = pred_nat * Rt (partition-bcast c==b)
  L = Ln(x+1e-12) (ACT), XL = x*L, E = sum_c XL, m = (exp(E)+1)*N
  xT_b = PE-transpose of x_nat[:, b, :] ; Y_b = xT_b * m-bcast
  M_ps[128,128] += Y_b^T @ xT_b  (512 accumulating matmuls)
Host: M = sum_cores(M_ps[0:64,0:64] + M_ps[64:128,64:128]); cov /= cov.sum(1);
loss = (cov.sum() - trace)/C.
"""

import numpy as np

B, C, W, H = 64, 64, 128, 128
NCORES = 8
WS = W // NCORES          # 16 w's per core
NPAIR = WS // 2           # 8 w-pairs per core
EPS = 1e-12

_CACHE = {}


def _build_nc():
    from contextlib import ExitStack

    import concourse.bass as bass
    import concourse.tile as tile
    from concourse import bacc, masks, mybir

    F32 = mybir.dt.float32
    BF16 = mybir.dt.bfloat16
    I32 = mybir.dt.int32
    AF = mybir.ActivationFunctionType
    OP = mybir.AluOpType

    nc = bacc.Bacc("TRN2", target_bir_lowering=False, debug=False)

    pred_t = nc.dram_tensor("pred", [B, C, WS, H], F32, kind="ExternalInput")
    gt_t = nc.dram_tensor("gt", [B, C, WS, H], I32, kind="ExternalInput")
    mout_t = nc.dram_tensor("m_out", [128, 128], F32, kind="ExternalOutput")

    # DRAM strides (elements) of the shard tensor (B, C, WS, H)
    SB_, SC_, SW_, SH_ = C * WS * H, WS * H, H, 1

    with tile.TileContext(nc) as tc, ExitStack() as ctx:
        singles = ctx.enter_context(tc.tile_pool(name="singles", bufs=1))
        pred_pool = ctx.enter_context(tc.tile_pool(name="pred", bufs=2))
        gt_pool = ctx.enter_context(tc.tile_pool(name="gt", bufs=2))
        x_pool = ctx.enter_context(tc.tile_pool(name="x", bufs=2))
        l_pool = ctx.enter_context(tc.tile_pool(name="l", bufs=2))
        xl_pool = ctx.enter_context(tc.tile_pool(name="xl", bufs=2))
        sm_pool = ctx.enter_context(tc.tile_pool(name="sm", bufs=3))
        yx_pool = ctx.enter_context(tc.tile_pool(name="yx", bufs=4))
        ps_dn = ctx.enter_context(tc.tile_pool(name="ps_dn", bufs=2, space="PSUM"))
        ps_er = ctx.enter_context(tc.tile_pool(name="ps_er", bufs=2, space="PSUM"))
        ps_xt = ctx.enter_context(tc.tile_pool(name="ps_xt", bufs=3, space="PSUM"))
        ps_m = ctx.enter_context(tc.tile_pool(name="ps_m", bufs=1, space="PSUM"))

        ident_b = singles.tile([128, 128], BF16)
        masks.make_identity(nc, ident_b[:])
        ident_f = singles.tile([128, 128], F32)
        masks.make_identity(nc, ident_f[:])
        ind = singles.tile([128, 2], BF16)
        nc.vector.memset(ind[:], 0.0)
        nc.vector.memset(ind[0:64, 0:1], 1.0)
        nc.vector.memset(ind[64:128, 1:2], 1.0)
        eps_t = singles.tile([128, 1], F32)
        nc.vector.memset(eps_t[:], EPS)

        m_ps = ps_m.tile([128, 128], F32)
        first_mm = [True]

        for wp in range(NPAIR):
            # ---- loads: [(j*64+c)=128p, b=64, h=128], w-major partitions ----
            def nat_in(t, j):
                return bass.AP(
                    tensor=t.ap().tensor,
                    offset=(wp * 2 + j) * SW_,
                    ap=[[SC_, 64], [SB_, 64], [SH_, H]],
                )

            pn = pred_pool.tile([128, 64, H], BF16)
            gn = gt_pool.tile([128, 64, H], BF16)
            for j in range(2):
                nc.gpsimd.dma_start(out=pn[j * 64:(j + 1) * 64], in_=nat_in(pred_t, j))
                nc.gpsimd.dma_start(out=gn[j * 64:(j + 1) * 64], in_=nat_in(gt_t, j))

            # ---- D / N via lhsT-trick ----
            dn = ps_dn.tile([128, 256], F32)
            Dv = dn[:, 0:128].rearrange("p (j b) -> p j b", j=2)
            Nv = dn[:, 128:256].rearrange("p (j b) -> p j b", j=2)
            for b in range(64):
                nc.tensor.matmul(Dv[:, :, b], pn[:, b, :], ind[:],
                                 start=True, stop=True, skip_group_check=True)
            for b in range(64):
                nc.tensor.matmul(Nv[:, :, b], gn[:, b, :], ind[:],
                                 start=True, stop=True, skip_group_check=True)

            # ---- R = 1/D, transpose to [(j*64+b), h], evac to bf16 ----
            rp = sm_pool.tile([128, 128], F32, tag="rp")
            nc.vector.reciprocal(rp[:], dn[:, 0:128])
            er = ps_er.tile([128, 256], F32)
            Ev = er[:, 0:128].rearrange("p (j b) -> p j b", j=2)
            rt_ps = er[:, 128:256]
            nc.tensor.matmul(rt_ps, rp[:], ident_f[:], is_transpose=True,
                             start=True, stop=True, skip_group_check=True)
            rt = sm_pool.tile([128, 128], BF16, tag="rt")
            nc.scalar.copy(rt[:], rt_ps)

            # ---- x = pred * Rt  (partition p=(j,c) reads Rt row (j,c): batch c) ----
            x = x_pool.tile([128, 64, H], BF16)
            rt_b = bass.AP(tensor=rt.tensor, offset=rt.offset,
                           ap=[rt.ap[0], [0, 64], [1, H]])
            nc.vector.tensor_mul(x[:], pn[:], rt_b)

            # ---- L = ln(x + eps); XL = x * L ----
            L = l_pool.tile([128, 64, H], BF16)
            nc.scalar.activation(L[:], x[:], AF.Ln, bias=eps_t[:], scale=1.0)
            xl = xl_pool.tile([128, 64, H], BF16)
            nc.vector.tensor_mul(xl[:], x[:], L[:])

            # ---- E = sum_c XL (lhsT-trick) ----
            for b in range(64):
                nc.tensor.matmul(Ev[:, :, b], xl[:, b, :], ind[:],
                                 start=True, stop=True, skip_group_check=True)

            # ---- m = (exp(E) + 1) * N ----
            expe = sm_pool.tile([128, 128], BF16, tag="expe")
            nc.scalar.activation(expe[:], er[:, 0:128], AF.Exp, bias=0.0, scale=1.0)
            mm = sm_pool.tile([128, 128], BF16, tag="m")
            nc.vector.scalar_tensor_tensor(
                out=mm[:], in0=expe[:], scalar=1.0, in1=dn[:, 128:256],
                op0=OP.add, op1=OP.mult,
            )

            # ---- transposes, Y, main MMs (spans of 8 b's) ----
            for sp in range(8):
                xt_ps = ps_xt.tile([128, 1024], BF16)
                for k in range(8):
                    b = sp * 8 + k
                    nc.tensor.matmul(xt_ps[:, k * 128:(k + 1) * 128], x[:, b, :],
                                     ident_b[:], is_transpose=True,
                                     start=True, stop=True, skip_group_check=True)
                y_sb = yx_pool.tile([128, 1024], BF16, tag="y")
                m_b = bass.AP(tensor=mm.tensor, offset=mm.offset + sp * 8,
                              ap=[mm.ap[0], [1, 8], [64, 2], [0, 64]])
                nc.vector.tensor_mul(y_sb[:], xt_ps[:], m_b)
                xt_sb = yx_pool.tile([128, 1024], BF16, tag="xt")
                nc.vector.tensor_copy(xt_sb[:], xt_ps[:])
                for k in range(8):
                    nc.tensor.matmul(
                        m_ps[:], y_sb[:, k * 128:(k + 1) * 128],
                        xt_sb[:, k * 128:(k + 1) * 128],
                        start=first_mm[0], stop=(wp == NPAIR - 1 and sp == 7 and k == 7),
                        skip_group_check=True,
                    )
                    first_mm[0] = False

        m_sb = singles.tile([128, 128], F32)
        nc.vector.tensor_copy(m_sb[:], m_ps[:])
        nc.sync.dma_start(out=mout_t.ap(), in_=m_sb[:])

    nc.compile()
    return nc


def _get_nc():
    if "nc" not in _CACHE:
        _CACHE["nc"] = _build_nc()
    return _CACHE["nc"]


def kernel(pred: np.ndarray, gt: np.ndarray) -> np.ndarray:
    from concourse.bass_utils import run_bass_kernel_spmd

    pred = np.ascontiguousarray(pred, dtype=np.float32)
    gt = np.ascontiguousarray(gt, dtype=np.int32)
    nc = _get_nc()

    in_maps = []
    for s in range(NCORES):
        in_maps.append({
            "pred": np.ascontiguousarray(pred[:, :, s * WS:(s + 1) * WS, :]),
            "gt": np.ascontiguousarray(gt[:, :, s * WS:(s + 1) * WS, :]),
        })
    res = run_bass_kernel_spmd(nc, in_maps, core_ids=list(range(NCORES)))

    M = np.zeros((64, 64), dtype=np.float32)
    for r in res.results:
        mo = r["m_out"]
        M += mo[0:64, 0:64] + mo[64:128, 64:128]
    cov = M / M.sum(axis=1)
    return np.float32((cov.sum() - np.trace(cov)) / C)



# revision 2
# speedup vs baseline: 1.3404x; 1.3404x over previous
"""Trainium2 Bass kernel for nn_ClassConfusionLoss.

Self-contained: takes FULL inputs pred (64,64,128,128) f32, gt (64,64,128,128) i32,
shards the spatial W axis across 8 NeuronCores, computes per-core partial weighted
covariance M (64x64), reduces on host and applies the final row-normalization +
trace (O(C^2), negligible).

Math: the reference's global scalars num_pos and S = sum(n*w_raw) scale cov by
alpha = num_pos/S, which cancels in cov / cov.sum(axis=1). So only
M[c,k] = sum_p n_p*w_raw_p*x_pc*x_pk is needed, where x[p,c] = pred[p,c]/D_p,
D_p = sum_c pred, n_p = sum_c(gt==1), w_raw = 1+exp(E), E = sum_c x ln x
= T/D - ln D with T = sum_c pred*ln(pred).

Pixel-major layout per core (w-slab of 16 = 4 w-quad tiles):
  tile [128p=(q,b), free=(c 64, j 2, h 128)] bf16, pixel w = 4t+2q+j.
  pred: 2 cast DMAs/tile with 512B descriptors (w-pair x h contiguous).
  n: 8 accumulate-DMAs/tile into n16[p,16,256] (same-address descriptors 16
  apart -> one DMA engine each -> race-free), folded 16->1 on DVE.
  D/T: packed bf16/fp16 add-trees on DVE; Ln/Exp/Sqrt on ACT.
  z = pred * sqrt(n*w_raw/D^2) in place; G += z_jh^T @ z_jh per h-slice
  (1024 accumulating 64x64 matmuls into one PSUM bank).
Host: M = sum_cores(G); cov = M/M.sum(0-axis semantics of ref); loss.
"""

import numpy as np

B, C, W, H = 64, 64, 128, 128
NCORES = 8
WS = W // NCORES          # 16 w's per core
NT = WS // 4              # 4 w-quad tiles per core
EPS = 1e-12

_CACHE = {}


def _build_nc():
    from contextlib import ExitStack

    import concourse.bass as bass
    import concourse.tile as tile
    from concourse import bacc, mybir

    F32 = mybir.dt.float32
    BF16 = mybir.dt.bfloat16
    FP16 = mybir.dt.float16
    I32 = mybir.dt.int32
    AF = mybir.ActivationFunctionType
    OP = mybir.AluOpType

    nc = bacc.Bacc("TRN2", target_bir_lowering=False, debug=False)

    pred_t = nc.dram_tensor("pred", [B, C, WS, H], F32, kind="ExternalInput")
    gt_t = nc.dram_tensor("gt", [B, C, WS, H], I32, kind="ExternalInput")
    mout_t = nc.dram_tensor("m_out", [64, 64], F32, kind="ExternalOutput")

    # DRAM strides (elements) of the shard tensor (B, C, WS, H)
    SB_, SC_, SW_ = C * WS * H, WS * H, H

    with tile.TileContext(nc) as tc, ExitStack() as ctx:
        singles = ctx.enter_context(tc.tile_pool(name="singles", bufs=1))
        pn_pool = ctx.enter_context(tc.tile_pool(name="pn", bufs=2))
        l_pool = ctx.enter_context(tc.tile_pool(name="l", bufs=2))
        d_pool = ctx.enter_context(tc.tile_pool(name="d", bufs=2))
        n_pool = ctx.enter_context(tc.tile_pool(name="n16", bufs=2))
        sm_pool = ctx.enter_context(tc.tile_pool(name="sm", bufs=2))
        ps_g = ctx.enter_context(tc.tile_pool(name="ps_g", bufs=1, space="PSUM"))

        eps_t = singles.tile([128, 1], F32)
        nc.vector.memset(eps_t[:], EPS)

        g_ps = ps_g.tile([64, 64], F32)

        for t in range(NT):
            # ---- pred load: [128p=(q,b), (c,j,h)] bf16, 512B descriptors ----
            pn = pn_pool.tile([128, 64, 256], BF16)
            for q in range(2):
                in_ap = bass.AP(tensor=pred_t.ap().tensor,
                                offset=(4 * t + 2 * q) * SW_,
                                ap=[[SB_, 64], [SC_, 64], [1, 256]])
                nc.gpsimd.dma_start(out=pn[64 * q:64 * (q + 1)], in_=in_ap)

            # ---- n: 16-way partial accumulate DMAs (i32 -> bf16 cast) ----
            n16 = n_pool.tile([128, 16, 256], BF16)
            for q in range(2):
                for ch in range(4):
                    in_ap = bass.AP(tensor=gt_t.ap().tensor,
                                    offset=(4 * t + 2 * q) * SW_ + 16 * ch * SC_,
                                    ap=[[SB_, 64], [SC_, 16], [1, 256]])
                    out_ap = bass.AP(tensor=n16.tensor,
                                     offset=n16.offset + 64 * q * n16.ap[0][0],
                                     ap=[[n16.ap[0][0], 64], [256, 16], [1, 256]])
                    nc.gpsimd.dma_start(
                        out=out_ap, in_=in_ap,
                        accum_op=(OP.bypass if ch == 0 else OP.add))

            # ---- N-tree: 16 -> 1 (bf16, exact for n<=64) ----
            n_bf = sm_pool.tile([128, 256], BF16, tag="n")
            nc.vector.tensor_tensor(out=n16[:, 0:8, :], in0=n16[:, 0:8, :],
                                    in1=n16[:, 8:16, :], op=OP.add)
            nc.vector.tensor_tensor(out=n16[:, 0:4, :], in0=n16[:, 0:4, :],
                                    in1=n16[:, 4:8, :], op=OP.add)
            nc.vector.tensor_tensor(out=n16[:, 0:2, :], in0=n16[:, 0:2, :],
                                    in1=n16[:, 2:4, :], op=OP.add)
            nc.vector.tensor_tensor(out=n_bf[:], in0=n16[:, 0, :],
                                    in1=n16[:, 1, :], op=OP.add)

            # ---- D-tree: sum_c pred (fp16 scratch, f32 final) ----
            dscr = d_pool.tile([128, 32, 256], FP16)
            d_f = sm_pool.tile([128, 256], F32, tag="d")
            nc.vector.tensor_tensor(out=dscr[:], in0=pn[:, 0:32, :],
                                    in1=pn[:, 32:64, :], op=OP.add)
            nc.vector.tensor_tensor(out=dscr[:, 0:16, :], in0=dscr[:, 0:16, :],
                                    in1=dscr[:, 16:32, :], op=OP.add)
            nc.vector.tensor_tensor(out=dscr[:, 0:8, :], in0=dscr[:, 0:8, :],
                                    in1=dscr[:, 8:16, :], op=OP.add)
            nc.vector.tensor_tensor(out=dscr[:, 0:4, :], in0=dscr[:, 0:4, :],
                                    in1=dscr[:, 4:8, :], op=OP.add)
            nc.vector.tensor_tensor(out=dscr[:, 0:2, :], in0=dscr[:, 0:2, :],
                                    in1=dscr[:, 2:4, :], op=OP.add)
            nc.vector.tensor_tensor(out=d_f[:], in0=dscr[:, 0, :],
                                    in1=dscr[:, 1, :], op=OP.add)

            # ---- L = ln(pred + eps); pl = pred * L (in place); T-tree ----
            L = l_pool.tile([128, 64, 256], FP16)
            nc.scalar.activation(L[:], pn[:], AF.Ln, bias=eps_t[:], scale=1.0)
            nc.vector.tensor_mul(L[:], pn[:], L[:])
            t_f = sm_pool.tile([128, 256], F32, tag="t")
            nc.vector.tensor_tensor(out=L[:, 0:32, :], in0=L[:, 0:32, :],
                                    in1=L[:, 32:64, :], op=OP.add)
            nc.vector.tensor_tensor(out=L[:, 0:16, :], in0=L[:, 0:16, :],
                                    in1=L[:, 16:32, :], op=OP.add)
            nc.vector.tensor_tensor(out=L[:, 0:8, :], in0=L[:, 0:8, :],
                                    in1=L[:, 8:16, :], op=OP.add)
            nc.vector.tensor_tensor(out=L[:, 0:4, :], in0=L[:, 0:4, :],
                                    in1=L[:, 4:8, :], op=OP.add)
            nc.vector.tensor_tensor(out=L[:, 0:2, :], in0=L[:, 0:2, :],
                                    in1=L[:, 2:4, :], op=OP.add)
            nc.vector.tensor_tensor(out=t_f[:], in0=L[:, 0, :],
                                    in1=L[:, 1, :], op=OP.add)

            # ---- per-pixel weight: rs = sqrt(n * (1 + exp(T/D - lnD)) / D^2) ----
            dr = sm_pool.tile([128, 256], F32, tag="dr")
            nc.vector.reciprocal(dr[:], d_f[:])
            lnd = sm_pool.tile([128, 256], F32, tag="lnd")
            nc.scalar.activation(lnd[:], d_f[:], AF.Ln, bias=eps_t[:], scale=1.0)
            e_f = sm_pool.tile([128, 256], F32, tag="e")
            nc.vector.tensor_mul(e_f[:], t_f[:], dr[:])
            nc.vector.tensor_tensor(out=e_f[:], in0=e_f[:], in1=lnd[:],
                                    op=OP.subtract)
            ee = sm_pool.tile([128, 256], F32, tag="ee")
            nc.scalar.activation(ee[:], e_f[:], AF.Exp, bias=0.0, scale=1.0)
            u_f = sm_pool.tile([128, 256], F32, tag="u")
            nc.vector.scalar_tensor_tensor(out=u_f[:], in0=ee[:], scalar=1.0,
                                           in1=n_bf[:], op0=OP.add, op1=OP.mult)
            nc.vector.tensor_mul(u_f[:], u_f[:], dr[:])
            nc.vector.tensor_mul(u_f[:], u_f[:], dr[:])
            rs = sm_pool.tile([128, 256], FP16, tag="rs")
            nc.scalar.activation(rs[:], u_f[:], AF.Sqrt, bias=0.0, scale=1.0)

            # ---- z = pred * rs (in place, rs broadcast over c) ----
            rs_b = bass.AP(tensor=rs.tensor, offset=rs.offset,
                           ap=[rs.ap[0], [0, 64], [1, 256]])
            nc.vector.tensor_mul(pn[:], pn[:], rs_b)

            # ---- G += z_jh^T @ z_jh per (j,h) slice ----
            for jh in range(256):
                z_ap = bass.AP(tensor=pn.tensor, offset=pn.offset + jh,
                               ap=[pn.ap[0], [256, 64]])
                nc.tensor.matmul(g_ps[:], z_ap, z_ap,
                                 start=(t == 0 and jh == 0),
                                 stop=(t == NT - 1 and jh == 255),
                                 skip_group_check=True)

        g_sb = singles.tile([64, 64], F32)
        nc.vector.tensor_copy(g_sb[:], g_ps[:])
        nc.sync.dma_start(out=mout_t.ap(), in_=g_sb[:])

    nc.compile()
    return nc


def _get_nc():
    if "nc" not in _CACHE:
        _CACHE["nc"] = _build_nc()
    return _CACHE["nc"]


def kernel(pred: np.ndarray, gt: np.ndarray) -> np.ndarray:
    from concourse.bass_utils import run_bass_kernel_spmd

    pred = np.ascontiguousarray(pred, dtype=np.float32)
    gt = np.ascontiguousarray(gt, dtype=np.int32)
    nc = _get_nc()

    in_maps = []
    for s in range(NCORES):
        in_maps.append({
            "pred": np.ascontiguousarray(pred[:, :, s * WS:(s + 1) * WS, :]),
            "gt": np.ascontiguousarray(gt[:, :, s * WS:(s + 1) * WS, :]),
        })
    res = run_bass_kernel_spmd(nc, in_maps, core_ids=list(range(NCORES)))

    M = np.zeros((64, 64), dtype=np.float64)
    for r in res.results:
        M += r["m_out"].astype(np.float64)
    cov = M / M.sum(axis=1)
    return np.float32((cov.sum() - np.trace(cov)) / C)
